# revision 2
# baseline (speedup 1.0000x reference)
"""Trainium2 Bass kernel v3 for the AttentionModel-without-residual problem.

Sharding: pure data parallel - batch 1024 split 128/core across 8 cores.

Key design points vs v2:
  - scores via transposed products + PE column-reduce: prodT = catT_s * cT
    (DVE/Pool f16), then 8 tiny [128cols x 1] matmuls per slot accumulate
    the per-batch dot into a persistent PSUM [128,21] score tile - the
    DVE/ACT reduction ops vanish.
  - score products split into h-halves and issued right after each c-half
    finalizes, overlapping the next step's score work with this step's
    gate-evacuation tail.
  - lin layer folded into dec_wih host-side (W2 = lin_w.T @ dec_wih.T,
    fp8 DR): the zm intermediate (matmul + evac + transpose + cast chain)
    is gone, and the h-path gate contributions pre-fill the gate PSUM
    quarters while scores/softmax/AV still run.
  - gate bias as an fp8 hi/lo DoubleRow pair (k=2) instead of an f16
    ones-row matmul: same precision class, half the PE cost.
  - cell state kept in f16 end-to-end (no f32 master + ACT copy).
  - h slots stored transposed (catT) as a side effect of the recurrence
    transposes; no DRAM staging of attention slots at all.
  - decoder output staged per step straight to DRAM.

Scales (power of two, exact; fp8e4m3 max finite is 224):
  S_ACT=128 (h/at fp8), S_W=1024 (dwhh/W2), S_EW=1024 (enc whh),
  gate PSUM scale S_G = S_ACT*S_W = 131072 = enc S_EG.
"""

import os
from contextlib import ExitStack

import ml_dtypes
import numpy as np

import concourse.bass as bass
import concourse.mybir as mybir
import concourse.tile as tile
from concourse.bass_utils import run_bass_kernel_spmd

BF16 = mybir.dt.bfloat16
F16 = mybir.dt.float16
F32 = mybir.dt.float32
FP8 = mybir.dt.float8e4
AX = mybir.AxisListType
ALU = mybir.AluOpType
AF = mybir.ActivationFunctionType
PM = mybir.MatmulPerfMode

H = 1024
D = 512
P = 66
T_IN = 10
T_OUT = 25
B = 1024
NCORES = 8
BS = B // NCORES
G4 = 4 * H
HALF = G4 // 2
QTR = G4 // 4
NMM = 512
NSLOT = 2 * T_IN + 1   # 21; block 20 = decoder h
NSTAT = NSLOT - 1      # 20 static slots in cat8/catT blocks 0..19

S_ACT = 128.0
S_W = 1024.0
S_G = S_ACT * S_W      # 131072
S_EW = 1024.0
S_EG = S_ACT * S_EW    # 131072
S_BH = 128.0           # bias hi multiplier (ones2 row 0)
S_BL = 8.0             # bias lo multiplier (ones2 row 1)
S_XT = 32.0            # encoder input fp8 scale
S_WX = S_EG / S_XT     # 4096, encoder input-proj weight fp8 scale
HIPRI = 50000          # priority offset for spine ops

_nf16 = np.float16
_nf8 = ml_dtypes.float8_e4m3

# product engine split per half: slot -> engine ('v' DVE, 'p' Pool)
PROD_POOL = {2, 5, 8, 11, 14, 17, 19}
# diag engine split: pair-slab ops on DVE for these (slot) indices
DIAG_DVE = {1, 3, 5, 7, 9, 11, 13, 15, 17, 19}


def _gate_perm(ngroup):
    """perm[new] = old gate-column index. ngroup groups, each [i|f|o|g]*w."""
    w = H // ngroup
    perm = np.empty(G4, dtype=np.int64)
    gate_old = [0, 1, 3, 2]  # i, f, o, g in torch i,f,g,o order
    u = np.arange(w)
    for j in range(ngroup):
        for q in range(4):
            new = j * 4 * w + q * w + u
            old = gate_old[q] * H + j * w + u
            perm[new] = old
    return perm


def _pack_pairs(wT, scale, perm):
    """[K, 4096] -> list of [128, 2, 4096] fp8 pair tensors (K=256 per pair)."""
    wp = wT[:, perm] * scale
    K = wT.shape[0]
    out = []
    for pair in range(K // 256):
        a = wp[pair * 256:(pair + 1) * 256]
        t = np.stack([a[0:128], a[128:256]], axis=1)
        out.append(np.ascontiguousarray(t.astype(_nf8)))
    return out


def _preprocess(inputs):
    f = {k: np.asarray(v, np.float64) for k, v in inputs.items()}
    hperm = _gate_perm(2)
    qperm = _gate_perm(4)

    def enc_pack(wih, bih, bhh):
        """fp8 DR pack of the input projection: 68 k-rows (66 x + bias hi/lo)
        as [34, 2, 4096]; slab 0 = rows 0..33, slab 1 = rows 34..67."""
        wx = f["wf"].T @ wih.T            # [66, 4096]
        brow = f["bf"] @ wih.T + bih + bhh
        wx8 = wx[:, hperm] * S_WX
        assert np.abs(wx8).max() < 224, f"wx fp8 overflow {np.abs(wx8).max()}"
        tgt = brow[hperm] * S_EG
        bh = (tgt / S_BH).astype(_nf8)
        resid = tgt - S_BH * bh.astype(np.float64)
        bl = (resid / S_BL).astype(_nf8)
        assert np.abs(tgt / S_BH).max() < 224, "enc bias hi overflow"
        assert np.abs(resid / S_BL).max() < 224, "enc bias lo overflow"
        wx68 = np.concatenate(
            [wx8.astype(_nf8).astype(np.float64), bh[None].astype(np.float64),
             bl[None].astype(np.float64)], axis=0)  # [68, 4096]
        t = np.stack([wx68[0:34], wx68[34:68]], axis=1)  # [34, 2, 4096]
        return np.ascontiguousarray(t.astype(_nf8))

    shared = {
        "enc_wx": enc_pack(f["enc_wih"], f["enc_bih"], f["enc_bhh"]),
        "encp_wx": enc_pack(f["encp_wih"], f["encp_bih"], f["encp_bhh"]),
        "pose_wT": np.ascontiguousarray(f["pose_w"].T.astype(_nf16)),
        "pose_b": np.ascontiguousarray(f["pose_b"][None].astype(_nf16)),
        "ident": np.ascontiguousarray(np.eye(128).astype(_nf16)),
        "identS": np.ascontiguousarray((np.eye(128) * S_ACT).astype(_nf16)),
        "ones_row": np.ascontiguousarray(np.ones((1, 128)).astype(_nf16)),
    }
    for i, t in enumerate(_pack_pairs(f["enc_whh"].T, S_EW, hperm)):
        shared[f"enc_whh8_{i}"] = t
    for i, t in enumerate(_pack_pairs(f["encp_whh"].T, S_EW, hperm)):
        shared[f"encp_whh8_{i}"] = t
    for i, t in enumerate(_pack_pairs(f["dec_whh"].T, S_W, qperm)):
        shared[f"dwhh8_{i}"] = t
    # W2 = lin folded into dec_wih: gates_h = hT @ (lin_w.T @ dec_wih.T)
    W2 = f["lin_w"].T @ f["dec_wih"].T  # [1024, 4096]
    for i, t in enumerate(_pack_pairs(W2, S_W, qperm)):
        shared[f"W28_{i}"] = t
    # bias as fp8 hi/lo DR pair: PSUM += S_BH*hi + S_BL*lo == S_G*dbias
    dbias = (f["dec_bih"] + f["dec_bhh"] + f["lin_b"] @ f["dec_wih"].T)[qperm]
    tgt = dbias * S_G
    hi = (tgt / S_BH).astype(_nf8)
    resid = tgt - S_BH * hi.astype(np.float64)
    lo = (resid / S_BL).astype(_nf8)
    assert np.abs(tgt / S_BH).max() < 224, "bias hi overflow"
    assert np.abs(resid / S_BL).max() < 224, "bias lo overflow"
    shared["dbias8"] = np.ascontiguousarray(np.stack([hi, lo])[None])  # [1,2,4096]
    ones2 = np.zeros((1, 2, 128))
    ones2[0, 0, :] = S_BH
    ones2[0, 1, :] = S_BL
    shared["ones2"] = np.ascontiguousarray(ones2.astype(_nf8))

    x = np.asarray(inputs["x"], np.float64)
    z = np.asarray(inputs["z"], np.float64)
    assert max(np.abs(x).max(), np.abs(z).max()) * S_XT < 224, "x fp8 overflow"
    per_core = []
    for c in range(NCORES):
        sl = slice(c * BS, (c + 1) * BS)

        def tr(a):
            at = a[sl].transpose(1, 2, 0) * S_XT  # [10, 66, 128]
            ext = np.concatenate(
                [at, np.full((T_IN, 1, BS), S_BH),
                 np.full((T_IN, 1, BS), S_BL)], axis=1)  # [10, 68, 128]
            # DR pair layout: [10, 34, 2, 128]
            return np.stack([ext[:, 0:34], ext[:, 34:68]], axis=2)

        xz = np.stack([tr(x), tr(z)], axis=0)  # [2, 10, 34, 2, 128]
        per_core.append(np.ascontiguousarray(xz.astype(_nf8)))
    return shared, per_core


def _emit(ctx, nc, tc, prm):
    cpool = ctx.enter_context(tc.tile_pool(name="cpool", bufs=1))

    # persistent state
    catT = cpool.tile([BS, NSLOT * H], F16)   # transposed slots, block 20 = h
    cat8 = cpool.tile([BS, NSTAT * H], FP8)   # untransposed static slots * S_ACT
    c_st = cpool.tile([BS, H], F16)           # enc/decoder cell state
    c2_st = cpool.tile([BS, H], F16)          # encp cell state (encoder only)
    h_un = cpool.tile([BS, H], F16)           # untransposed decoder h
    cT = cpool.tile([BS, H], F16)             # transposed decoder c
    ones_col = cpool.tile([128, 1], F16)
    nc.vector.memset(ones_col[:], 1.0)

    hT8_final = None  # fp8 S_ACT*hT of enc t=9, feeds decoder step 0

    # ------------- encoder phase (enc and encp interleaved) ----------------
    with ExitStack() as ph:
        ewp = ph.enter_context(tc.tile_pool(name="ewp", bufs=1))
        gp = ph.enter_context(tc.tile_pool(name="gpe", bufs=1, space="PSUM"))
        sc = ph.enter_context(tc.tile_pool(name="sce", bufs=2))

        cfg = []
        for li, (wx_n, whh_n, cst) in enumerate(
            [("enc_wx", "enc_whh8", c_st), ("encp_wx", "encp_whh8", c2_st)]
        ):
            wx = ewp.tile([34, 2, G4], FP8, tag=f"wx{li}")
            nc.sync.dma_start(out=wx[:], in_=prm[wx_n][:])
            whh = []
            for k in range(4):
                t = ewp.tile([128, 2, G4], FP8, tag=f"ewhh{li}_{k}")
                nc.sync.dma_start(out=t[:], in_=prm[f"{whh_n}_{k}"][:])
                whh.append(t)
            xt = ewp.tile([34, T_IN * 2 * 128], FP8, tag=f"xt{li}")
            nc.sync.dma_start(
                out=xt[:].rearrange("p (t k b) -> p t k b", t=T_IN, k=2),
                in_=prm["xzT"][li].rearrange("t p k b -> p t k b"),
            )
            cfg.append((wx, whh, xt, cst))
        xt3 = [cfg[li][2][:].rearrange("p (t k b) -> p t k b", t=T_IN, k=2)
               for li in range(2)]

        h8prev = [None, None]
        hlast = [None, None]
        for t in range(T_IN):
            htiles = [
                sc.tile([BS, H], F16, tag=f"he{li}", name=f"he{li}_{t}")
                for li in range(2)
            ]
            h8new = [
                cpool.tile([BS, H], FP8, name="h8_dec")
                if (t == T_IN - 1 and li == 0)
                else sc.tile([BS, H], FP8, tag=f"h8{li}", name=f"h8{li}_{t}")
                for li in range(2)
            ]
            for j in range(2):  # gate halves [i|f|o|g]*512
                for li, (wx, whh, xt, cst) in enumerate(cfg):
                    skew = False
                    with tc.tile_wait_until(0.005, enable=skew):
                        ps = gp.tile([BS, HALF], F32, tag=f"g{li}",
                                     name=f"eg{li}_{t}_{j}")
                        for n in range(HALF // NMM):
                            co = j * HALF + n * NMM
                            nc.tensor.matmul(
                                ps[:, n * NMM:(n + 1) * NMM],
                                xt3[li][:, t, :, :],
                                wx[:, :, co:co + NMM],
                                start=True, stop=(t == 0),
                                perf_mode=PM.DoubleRow,
                            )
                            if t > 0:
                                h38 = h8prev[li][:].rearrange(
                                    "p (k b) -> p k b", k=8)
                                for kp in range(4):
                                    nc.tensor.matmul(
                                        ps[:, n * NMM:(n + 1) * NMM],
                                        h38[:, 2 * kp:2 * kp + 2, :],
                                        whh[kp][:, :, co:co + NMM],
                                        start=False, stop=(kp == 3),
                                        perf_mode=PM.DoubleRow,
                                    )
                        # evacuate half: [i|f|o|g]*512
                        hs = slice(512 * j, 512 * (j + 1))
                        slot = li * T_IN + t
                        scol = slot * H + 512 * j
                        with tc.high_priority(HIPRI):
                            sif = sc.tile([BS, 1536], F16, tag="esif")
                            tg = sc.tile([BS, 512], F16, tag="etg")
                            nc.scalar.activation(sif[:], ps[:, 0:1536],
                                                 AF.Sigmoid, scale=1.0 / S_EG)
                            nc.scalar.activation(tg[:], ps[:, 1536:2048],
                                                 AF.Tanh, scale=1.0 / S_EG)
                            if t == 0:
                                nc.vector.tensor_mul(cst[:, hs], sif[:, 0:512],
                                                     tg[:])
                            else:
                                t2 = sc.tile([BS, 512], F16, tag="et2")
                                nc.vector.tensor_mul(t2[:], sif[:, 0:512], tg[:])
                                t1 = sc.tile([BS, 512], F16, tag="et1")
                                nc.gpsimd.tensor_mul(t1[:], sif[:, 512:1024],
                                                     cst[:, hs])
                                nc.vector.tensor_add(cst[:, hs], t1[:], t2[:])
                            tc_ = sc.tile([BS, 512], F16, tag="etc")
                            nc.scalar.activation(tc_[:], cst[:, hs], AF.Tanh)
                            h_cols = htiles[li][:, hs]
                            nc.vector.tensor_mul(h_cols, sif[:, 1024:1536], tc_[:])
                            nc.sync.dma_start_transpose(
                                catT[:, scol:scol + 512].rearrange(
                                    "p (k b) -> p k b", k=4),
                                h_cols,
                            )
                            nc.vector.tensor_scalar_mul(
                                h8new[li][:, hs], catT[:, scol:scol + 512],
                                S_ACT)
                        nc.gpsimd.tensor_scalar_mul(
                            cat8[:, scol:scol + 512], h_cols, S_ACT)
            for li in range(2):
                h8prev[li] = h8new[li]
                hlast[li] = htiles[li]

        # decoder primes: h = enc h_9; catT block 20 = catT block 9
        nc.vector.tensor_copy(h_un[:], hlast[0][:])
        nc.vector.tensor_copy(
            catT[:, NSTAT * H:(NSTAT + 1) * H],
            catT[:, (T_IN - 1) * H:T_IN * H],
        )
        hT8_final = h8prev[0]

    # ---------------- decoder phase ---------------------------------------
    with ExitStack() as ph:
        dwp = ph.enter_context(tc.tile_pool(name="dwp", bufs=1))
        ident = dwp.tile([128, 128], F16)
        identS = dwp.tile([128, 128], F16)
        ones_row = dwp.tile([1, 128], F16)
        ones2 = dwp.tile([1, 2, 128], FP8)
        dbias8 = dwp.tile([1, 2, G4], FP8)
        nc.sync.dma_start(out=ident[:], in_=prm["ident"][:])
        nc.sync.dma_start(out=identS[:], in_=prm["identS"][:])
        nc.sync.dma_start(out=ones_row[:], in_=prm["ones_row"][:])
        nc.sync.dma_start(out=ones2[:], in_=prm["ones2"][:])
        nc.sync.dma_start(out=dbias8[:], in_=prm["dbias8"][:])
        dwhh8 = []
        W28 = []
        for i in range(4):
            t = dwp.tile([128, 2, G4], FP8, tag=f"dwhh{i}")
            nc.sync.dma_start(out=t[:], in_=prm[f"dwhh8_{i}"][:])
            dwhh8.append(t)
            t = dwp.tile([128, 2, G4], FP8, tag=f"W28{i}")
            nc.sync.dma_start(out=t[:], in_=prm[f"W28_{i}"][:])
            W28.append(t)
        posew = []
        for k in range(8):
            t = dwp.tile([128, P], F16, tag=f"pose{k}")
            nc.sync.dma_start(out=t[:], in_=prm["pose_wT"][k * 128:(k + 1) * 128, :])
            posew.append(t)
        pbias = dwp.tile([1, P], F16)
        nc.sync.dma_start(out=pbias[:], in_=prm["pose_b"][:])

        sc = ph.enter_context(tc.tile_pool(name="scd", bufs=2))
        s1 = ph.enter_context(tc.tile_pool(name="s1d", bufs=2))
        prd = ph.enter_context(tc.tile_pool(name="prd", bufs=8))
        prp = ph.enter_context(tc.tile_pool(name="prp", bufs=6))
        att = ph.enter_context(tc.tile_pool(name="att", bufs=3))
        gq = ph.enter_context(tc.tile_pool(name="gq", bufs=2, space="PSUM"))
        atp = ph.enter_context(tc.tile_pool(name="atp", bufs=1, space="PSUM"))
        spp = ph.enter_context(tc.tile_pool(name="spp", bufs=1, space="PSUM"))
        pop = ph.enter_context(tc.tile_pool(name="pop", bufs=1, space="PSUM"))

        scps = spp.tile([BS, NSLOT], F32)
        po_ps = pop.tile([BS, P], F32)

        def emit_products(col0, ncols, n_dve, tagsuf):
            """Score products over h-cols [col0, col0+ncols); first n_dve
            slots (mod rotation) on DVE, rest on Pool."""
            out = []
            for s in range(NSLOT):
                src = catT[:, s * H + col0: s * H + col0 + ncols]
                dve = ((s + col0 // 256) % NSLOT) < n_dve
                if dve:
                    pt = prd.tile([128, ncols], F16, tag=f"ptd{ncols}",
                                  name=f"ptd_{tagsuf}_{s}")
                    nc.vector.tensor_mul(pt[:], src, cT[:, col0:col0 + ncols])
                else:
                    pt = prp.tile([128, ncols], F16, tag=f"ptp{ncols}",
                                  name=f"ptp_{tagsuf}_{s}")
                    nc.gpsimd.tensor_mul(pt[:], src, cT[:, col0:col0 + ncols])
                out.append((s, col0, ncols, pt))
            return out

        def emit_reduces(prods):
            for s, col0, ncols, pt in prods:
                for sl in range(ncols // 128):
                    slab = col0 // 128 + sl
                    nc.tensor.matmul(
                        scps[:][:, s:s + 1],
                        pt[:, sl * 128:(sl + 1) * 128],
                        ones_col[:],
                        start=(slab == 0), stop=(slab == 7),
                    )

        # ---- prime: cT + scores for step 0 from the encoder-final state ----
        for q in range(4):
            qs = slice(256 * q, 256 * (q + 1))
            nc.sync.dma_start_transpose(
                cT[:, qs].rearrange("p (k b) -> p k b", k=2), c_st[:, qs])
        for hf in range(2):
            prods = emit_products(hf * 512, 512, 10, f"pr{hf}")
            emit_reduces(prods)

        hT8 = hT8_final
        for step in range(T_OUT):
            # ---- PE: prefill gate chunks (bias + W2 from hT8) ----
            # per quarter two 512-col 1-bank PSUM chunks: A=[i|f]*256 B=[o|g]*256
            gps = []
            def prefill(q):
                h38 = hT8[:].rearrange("p (k b) -> p k b", k=8)
                ps = gq.tile([BS, QTR], F32, tag="gq", name=f"gq_{step}_{q}")
                for n in range(2):
                    co = q * QTR + n * NMM
                    osl = slice(n * NMM, (n + 1) * NMM)
                    nc.tensor.matmul(
                        ps[:, osl], ones2[:], dbias8[:, :, co:co + NMM],
                        start=True, stop=False, perf_mode=PM.DoubleRow,
                    )
                    for kp in range(4):
                        nc.tensor.matmul(
                            ps[:, osl],
                            h38[:, 2 * kp:2 * kp + 2, :],
                            W28[kp][:, :, co:co + NMM],
                            start=False, stop=False, perf_mode=PM.DoubleRow,
                        )
                return ps

            gps.append(prefill(0))
            gps.append(prefill(1))

            # ---- softmax over scps (scores are small: no max-sub needed) ----
            with tc.high_priority(HIPRI):
                num = s1.tile([BS, NSLOT], F32, tag="num")
                den = s1.tile([BS, NSLOT], F32, tag="den")
                nc.scalar.activation(num[:], scps[:], AF.Sigmoid)
                nc.scalar.activation(den[:], scps[:], AF.Sigmoid, scale=-1.0)
                rden = s1.tile([BS, NSLOT], F32, tag="rden")
                nc.vector.reciprocal(rden[:], den[:])
                wts = s1.tile([BS, NSLOT], F32, tag="wts")
                nc.vector.tensor_mul(wts[:], num[:], rden[:])
                wjunk = s1.tile([BS, NSLOT], F32, tag="wjunk")
                ssum = s1.tile([BS, 1], F32, tag="ssum")
                nc.vector.tensor_scalar(
                    out=wjunk[:], in0=wts[:], scalar1=1.0, scalar2=0.0,
                    op0=ALU.mult, op1=ALU.add, accum_out=ssum[:],
                )
                rcp = s1.tile([BS, 1], F32, tag="rcp")
                nc.vector.reciprocal(rcp[:], ssum[:])
                wn = s1.tile([BS, NSLOT], F32, tag="wn")
                nc.vector.tensor_scalar_mul(wn[:], wts[:], rcp[:])

                # ---- diag weights ----
                dg2s = []
                for p_i in range(10):
                    dg2 = att.tile([128, 2, 128], FP8, tag=f"dg2_{p_i}",
                                   name=f"dg2_{step}_{p_i}")
                    for half in range(2):
                        s = 2 * p_i + half
                        eng = nc.vector if s in DIAG_DVE else nc.gpsimd
                        eng.tensor_scalar_mul(dg2[:, half, :], ident[:],
                                              wn[:, s:s + 1])
                    dg2s.append(dg2)
                dg10 = att.tile([128, 128], F16, tag="dg10", name=f"dg10_{step}")
                nc.gpsimd.tensor_scalar_mul(dg10[:], identS[:],
                                            wn[:, NSTAT:NSLOT])

            # ---- AV transposed: atT_ps[h128, b] += cat8(moving) x dg(stat) --
            atT8 = s1.tile([BS, H], FP8, tag="atT8")
            c83 = cat8[:].rearrange("p (s h) -> p s h", s=NSTAT)
            atps = atp.tile([BS, H], F32, tag="at", name=f"at_{step}")
            for hb in range(8):
                hsl = slice(hb * 128, (hb + 1) * 128)
                for p_i in range(10):
                    nc.tensor.matmul(
                        atps[:, hsl],
                        c83[:, 2 * p_i:2 * p_i + 2, hsl],
                        dg2s[p_i][:],
                        start=(p_i == 0), stop=False,
                        perf_mode=PM.DoubleRow,
                    )
                nc.tensor.matmul(
                    atps[:, hsl], h_un[:, hsl], dg10[:],
                    start=False, stop=True,
                )
                if hb == 3:
                    with tc.high_priority(HIPRI):
                        nc.scalar.activation(atT8[:, 0:512], atps[:, 0:512],
                                             AF.Copy)
                elif hb == 7:
                    with tc.high_priority(HIPRI):
                        nc.scalar.activation(atT8[:, 512:1024],
                                             atps[:, 512:1024], AF.Copy)

            # ---- gates: 4 PSUM quarters [i|f|o|g]*256 ----
            a3 = atT8[:].rearrange("p (k b) -> p k b", k=8)
            hT8_new = att.tile([BS, H], FP8, tag="hT8n", name=f"hT8_{step}")
            prods0 = prods1 = None

            def at_dr(q):
                ps = gps[q]
                for n in range(2):
                    co = q * QTR + n * NMM
                    osl = slice(n * NMM, (n + 1) * NMM)
                    for kp in range(4):
                        nc.tensor.matmul(
                            ps[:, osl],
                            a3[:, 2 * kp:2 * kp + 2, :],
                            dwhh8[kp][:, :, co:co + NMM],
                            start=False, stop=(kp == 3),
                            perf_mode=PM.DoubleRow,
                        )

            def evac(q):
                ps = gps[q]
                qs = slice(256 * q, 256 * (q + 1))
                with tc.high_priority(HIPRI):
                    sif = sc.tile([BS, 768], F16, tag="sif")
                    tg = sc.tile([BS, 256], F16, tag="tg")
                    nc.scalar.activation(sif[:], ps[:, 0:768], AF.Sigmoid,
                                         scale=1.0 / S_G)
                    nc.scalar.activation(tg[:], ps[:, 768:1024], AF.Tanh,
                                         scale=1.0 / S_G)
                    t2 = sc.tile([BS, 256], F16, tag="t2")
                    nc.gpsimd.tensor_mul(t2[:], sif[:, 0:256], tg[:])
                    t1 = sc.tile([BS, 256], F16, tag="t1")
                    nc.vector.tensor_mul(t1[:], sif[:, 256:512], c_st[:, qs])
                    nc.vector.tensor_add(c_st[:, qs], t1[:], t2[:])
                    tc_ = sc.tile([BS, 256], F16, tag="tc")
                    nc.scalar.activation(tc_[:], c_st[:, qs], AF.Tanh)
                    nc.vector.tensor_mul(h_un[:, qs], sif[:, 512:768], tc_[:])
                    hcol = NSTAT * H + 256 * q
                    nc.sync.dma_start_transpose(
                        catT[:, hcol:hcol + 256].rearrange(
                            "p (k b) -> p k b", k=2),
                        h_un[:, qs],
                    )
                    nc.sync.dma_start_transpose(
                        cT[:, qs].rearrange("p (k b) -> p k b", k=2),
                        c_st[:, qs])

            at_dr(0)
            evac(0)
            at_dr(1)
            evac(1)
            # half0 of next scores + hT8 half0
            nc.vector.tensor_scalar_mul(
                hT8_new[:, 0:512], catT[:, NSTAT * H:NSTAT * H + 512], S_ACT)
            prods0 = emit_products(0, 512, 14, f"s{step}h0")
            gps.append(prefill(2))
            at_dr(2)
            evac(2)
            gps.append(prefill(3))
            at_dr(3)
            evac(3)
            nc.gpsimd.tensor_scalar_mul(
                hT8_new[:, 512:1024],
                catT[:, NSTAT * H + 512:NSTAT * H + 1024], S_ACT)
            prods3 = emit_products(512, 512, 14, f"s{step}h1")
            hT8 = hT8_new

            # ---- pose output ----
            nc.tensor.matmul(po_ps[:], ones_row[:], pbias[:], start=True,
                             stop=False)
            for k in range(8):
                nc.tensor.matmul(
                    po_ps[:], catT[:, NSTAT * H + k * 128:NSTAT * H + (k + 1) * 128],
                    posew[k][:], start=False, stop=(k == 7),
                )
            stg = s1.tile([BS, P], F32, tag="stg")
            nc.scalar.activation(stg[:], po_ps[:], AF.Copy)
            nc.sync.dma_start(out=prm["out"][:, step, :], in_=stg[:])

            # ---- PE score reduces (tail) ----
            emit_reduces(prods0)
            emit_reduces(prods3)


_WAIT_LIMIT = {}
_WAIT_LIMIT_DEFAULT = 1


def _fix_waits(nc):
    """Hoist excess sync waits onto InstEventSemaphore carriers (the hardware
    instruction structs accept a limited number of wait commands)."""
    wid = 0
    for blk in nc.m.functions[0].blocks:
        insts = list(blk.instructions)
        out = []
        changed = False
        for inst in insts:
            si = getattr(inst, "sync_info", None)
            limit = _WAIT_LIMIT.get(type(inst).__name__, _WAIT_LIMIT_DEFAULT)
            if si is not None and len(si.on_wait) > limit:
                keep = si.on_wait[-limit:] if limit else []
                hoist = si.on_wait[: len(si.on_wait) - limit]
                for w in hoist:
                    carrier = mybir.InstEventSemaphore(
                        name=f"WFIX-{wid}",
                        engine=inst.engine,
                        ins=[],
                        outs=[],
                        sync_info=mybir.SyncInfo(on_wait=[w], on_update=[]),
                    )
                    wid += 1
                    out.append(carrier)
                inst.sync_info = mybir.SyncInfo(
                    on_wait=keep, on_update=list(si.on_update)
                )
                changed = True
            out.append(inst)
        if changed:
            blk.instructions = out


def build_nc(fix_waits=True):
    nc = bass.Bass()
    prm = {}
    decls = [
        ("ident", [128, 128], F16),
        ("identS", [128, 128], F16),
        ("ones_row", [1, 128], F16),
        ("ones2", [1, 2, 128], FP8),
        ("dbias8", [1, 2, G4], FP8),
        ("xzT", [2, T_IN, 34, 2, BS], FP8),
        ("enc_wx", [34, 2, G4], FP8),
        ("encp_wx", [34, 2, G4], FP8),
        ("pose_wT", [H, P], F16),
        ("pose_b", [1, P], F16),
    ]
    for i in range(4):
        decls.append((f"enc_whh8_{i}", [128, 2, G4], FP8))
        decls.append((f"encp_whh8_{i}", [128, 2, G4], FP8))
        decls.append((f"dwhh8_{i}", [128, 2, G4], FP8))
        decls.append((f"W28_{i}", [128, 2, G4], FP8))
    for name, shape, dt in decls:
        prm[name] = nc.declare_dram_parameter(name, shape, dt, isOutput=False)
    prm["out"] = nc.declare_dram_parameter("out", [BS, T_OUT, P], F32,
                                           isOutput=True)

    with ExitStack() as ctx:
        tc = ctx.enter_context(tile.TileContext(nc))
        _emit(ctx, nc, tc, prm)
    if fix_waits:
        _fix_waits(nc)
    return nc


def make_in_maps(inputs):
    shared, per_core = _preprocess(inputs)
    in_maps = []
    for c in range(NCORES):
        m = dict(shared)
        m["xzT"] = per_core[c]
        in_maps.append(m)
    return in_maps


def run(inputs, **kw):
    nc = build_nc()
    in_maps = make_in_maps(inputs)
    return run_bass_kernel_spmd(nc, in_maps, list(range(NCORES)), **kw)


def kernel(**inputs) -> np.ndarray:
    res = run(inputs)
    return np.concatenate(
        [res.results[c]["out"] for c in range(NCORES)], axis=0
    ).astype(np.float32)


if __name__ == "__main__":
    nc = build_nc()
    print("built ok")


# revision 8
# speedup vs baseline: 1.0346x; 1.0346x over previous
"""Trainium2 Bass kernel v3 for the AttentionModel-without-residual problem.

Sharding: pure data parallel - batch 1024 split 128/core across 8 cores.

Key design points vs v2:
  - scores via transposed products + PE column-reduce: prodT = catT_s * cT
    (DVE/Pool f16), then 8 tiny [128cols x 1] matmuls per slot accumulate
    the per-batch dot into a persistent PSUM [128,21] score tile - the
    DVE/ACT reduction ops vanish.
  - score products split into h-halves and issued right after each c-half
    finalizes, overlapping the next step's score work with this step's
    gate-evacuation tail.
  - lin layer folded into dec_wih host-side (W2 = lin_w.T @ dec_wih.T,
    fp8 DR): the zm intermediate (matmul + evac + transpose + cast chain)
    is gone, and the h-path gate contributions pre-fill the gate PSUM
    quarters while scores/softmax/AV still run.
  - gate bias as an fp8 hi/lo DoubleRow pair (k=2) instead of an f16
    ones-row matmul: same precision class, half the PE cost.
  - cell state kept in f16 end-to-end (no f32 master + ACT copy).
  - h slots stored transposed (catT) as a side effect of the recurrence
    transposes; no DRAM staging of attention slots at all.
  - decoder output staged per step straight to DRAM.

Scales (power of two, exact; fp8e4m3 max finite is 224):
  S_ACT=128 (h/at fp8), S_W=1024 (dwhh/W2), S_EW=1024 (enc whh),
  gate PSUM scale S_G = S_ACT*S_W = 131072 = enc S_EG.
"""

import os
from contextlib import ExitStack

import ml_dtypes
import numpy as np

import concourse.bass as bass
import concourse.mybir as mybir
import concourse.tile as tile
from concourse.bass_utils import run_bass_kernel_spmd

BF16 = mybir.dt.bfloat16
F16 = mybir.dt.float16
F32 = mybir.dt.float32
FP8 = mybir.dt.float8e4
AX = mybir.AxisListType
ALU = mybir.AluOpType
AF = mybir.ActivationFunctionType
PM = mybir.MatmulPerfMode

H = 1024
D = 512
P = 66
T_IN = 10
T_OUT = 25
B = 1024
NCORES = 8
BS = B // NCORES
G4 = 4 * H
HALF = G4 // 2
QTR = G4 // 4
NMM = 512
NSLOT = 2 * T_IN + 1   # 21; block 20 = decoder h
NSTAT = NSLOT - 1      # 20 static slots in cat8/catT blocks 0..19

S_ACT = 128.0
S_W = 1024.0
S_G = S_ACT * S_W      # 131072
S_EW = 1024.0
S_EG = S_ACT * S_EW    # 131072
S_BH = 128.0           # bias hi multiplier (ones2 row 0)
S_BL = 8.0             # bias lo multiplier (ones2 row 1)
S_XT = 32.0            # encoder input fp8 scale
S_WX = S_EG / S_XT     # 4096, encoder input-proj weight fp8 scale
HIPRI = 50000          # priority offset for spine ops

_nf16 = np.float16
_nf8 = ml_dtypes.float8_e4m3

# product engine split per half: slot -> engine ('v' DVE, 'p' Pool)
PROD_POOL = {2, 5, 8, 11, 14, 17, 19}
# diag engine split: pair-slab ops on DVE for these (slot) indices
DIAG_DVE = {1, 3, 5, 7, 9, 11, 13, 15, 17, 19}


def _gate_perm(ngroup):
    """perm[new] = old gate-column index. ngroup groups, each [i|f|o|g]*w."""
    w = H // ngroup
    perm = np.empty(G4, dtype=np.int64)
    gate_old = [0, 1, 3, 2]  # i, f, o, g in torch i,f,g,o order
    u = np.arange(w)
    for j in range(ngroup):
        for q in range(4):
            new = j * 4 * w + q * w + u
            old = gate_old[q] * H + j * w + u
            perm[new] = old
    return perm


def _pack_pairs(wT, scale, perm):
    """[K, 4096] -> list of [128, 2, 4096] fp8 pair tensors (K=256 per pair)."""
    wp = wT[:, perm] * scale
    K = wT.shape[0]
    out = []
    for pair in range(K // 256):
        a = wp[pair * 256:(pair + 1) * 256]
        t = np.stack([a[0:128], a[128:256]], axis=1)
        out.append(np.ascontiguousarray(t.astype(_nf8)))
    return out


def _preprocess(inputs):
    f = {k: np.asarray(v, np.float64) for k, v in inputs.items()}
    hperm = _gate_perm(2)
    qperm = _gate_perm(4)

    def enc_pack(wih, bih, bhh):
        """fp8 DR pack of the input projection: 68 k-rows (66 x + bias hi/lo)
        as [34, 2, 4096]; slab 0 = rows 0..33, slab 1 = rows 34..67."""
        wx = f["wf"].T @ wih.T            # [66, 4096]
        brow = f["bf"] @ wih.T + bih + bhh
        wx8 = wx[:, hperm] * S_WX
        assert np.abs(wx8).max() < 224, f"wx fp8 overflow {np.abs(wx8).max()}"
        tgt = brow[hperm] * S_EG
        bh = (tgt / S_BH).astype(_nf8)
        resid = tgt - S_BH * bh.astype(np.float64)
        bl = (resid / S_BL).astype(_nf8)
        assert np.abs(tgt / S_BH).max() < 224, "enc bias hi overflow"
        assert np.abs(resid / S_BL).max() < 224, "enc bias lo overflow"
        wx68 = np.concatenate(
            [wx8.astype(_nf8).astype(np.float64), bh[None].astype(np.float64),
             bl[None].astype(np.float64)], axis=0)  # [68, 4096]
        t = np.stack([wx68[0:34], wx68[34:68]], axis=1)  # [34, 2, 4096]
        return np.ascontiguousarray(t.astype(_nf8))

    shared = {
        "enc_wx": enc_pack(f["enc_wih"], f["enc_bih"], f["enc_bhh"]),
        "encp_wx": enc_pack(f["encp_wih"], f["encp_bih"], f["encp_bhh"]),
        "pose_wT": np.ascontiguousarray(f["pose_w"].T.astype(_nf16)),
        "pose_b": np.ascontiguousarray(f["pose_b"][None].astype(_nf16)),
        "ident": np.ascontiguousarray(np.eye(128).astype(_nf16)),
        "identS": np.ascontiguousarray((np.eye(128) * S_ACT).astype(_nf16)),
        "ones_row": np.ascontiguousarray(np.ones((1, 128)).astype(_nf16)),
    }
    for i, t in enumerate(_pack_pairs(f["enc_whh"].T, S_EW, hperm)):
        shared[f"enc_whh8_{i}"] = t
    for i, t in enumerate(_pack_pairs(f["encp_whh"].T, S_EW, hperm)):
        shared[f"encp_whh8_{i}"] = t
    for i, t in enumerate(_pack_pairs(f["dec_whh"].T, S_W, qperm)):
        shared[f"dwhh8_{i}"] = t
    # W2 = lin folded into dec_wih: gates_h = hT @ (lin_w.T @ dec_wih.T)
    W2 = f["lin_w"].T @ f["dec_wih"].T  # [1024, 4096]
    for i, t in enumerate(_pack_pairs(W2, S_W, qperm)):
        shared[f"W28_{i}"] = t
    # bias as fp8 hi/lo DR pair: PSUM += S_BH*hi + S_BL*lo == S_G*dbias
    dbias = (f["dec_bih"] + f["dec_bhh"] + f["lin_b"] @ f["dec_wih"].T)[qperm]
    tgt = dbias * S_G
    hi = (tgt / S_BH).astype(_nf8)
    resid = tgt - S_BH * hi.astype(np.float64)
    lo = (resid / S_BL).astype(_nf8)
    assert np.abs(tgt / S_BH).max() < 224, "bias hi overflow"
    assert np.abs(resid / S_BL).max() < 224, "bias lo overflow"
    shared["dbias8"] = np.ascontiguousarray(np.stack([hi, lo])[None])  # [1,2,4096]
    ones2 = np.zeros((1, 2, 128))
    ones2[0, 0, :] = S_BH
    ones2[0, 1, :] = S_BL
    shared["ones2"] = np.ascontiguousarray(ones2.astype(_nf8))

    x = np.asarray(inputs["x"], np.float64)
    z = np.asarray(inputs["z"], np.float64)
    assert max(np.abs(x).max(), np.abs(z).max()) * S_XT < 224, "x fp8 overflow"
    per_core = []
    for c in range(NCORES):
        sl = slice(c * BS, (c + 1) * BS)

        def tr(a):
            at = a[sl].transpose(1, 2, 0) * S_XT  # [10, 66, 128]
            ext = np.concatenate(
                [at, np.full((T_IN, 1, BS), S_BH),
                 np.full((T_IN, 1, BS), S_BL)], axis=1)  # [10, 68, 128]
            # DR pair layout: [10, 34, 2, 128]
            return np.stack([ext[:, 0:34], ext[:, 34:68]], axis=2)

        xz = np.stack([tr(x), tr(z)], axis=0)  # [2, 10, 34, 2, 128]
        per_core.append(np.ascontiguousarray(xz.astype(_nf8)))
    return shared, per_core


def _emit(ctx, nc, tc, prm):
    cpool = ctx.enter_context(tc.tile_pool(name="cpool", bufs=1))

    # persistent state
    catT = cpool.tile([BS, NSLOT * H], F16)   # transposed slots, block 20 = h
    cat8 = cpool.tile([BS, NSTAT * H], FP8)   # untransposed static slots * S_ACT
    c_st = cpool.tile([BS, H], F16)           # enc/decoder cell state
    c2_st = cpool.tile([BS, H], F16)          # encp cell state (encoder only)
    h_un = cpool.tile([BS, H], F16)           # untransposed decoder h
    cT = cpool.tile([BS, H], F16)             # transposed decoder c
    ones_col = cpool.tile([128, 1], F16)
    nc.vector.memset(ones_col[:], 1.0)

    hT8_final = None  # fp8 S_ACT*hT of enc t=9, feeds decoder step 0

    # ------------- encoder phase (enc and encp interleaved) ----------------
    with ExitStack() as ph:
        ewp = ph.enter_context(tc.tile_pool(name="ewp", bufs=1))
        gp = ph.enter_context(tc.tile_pool(name="gpe", bufs=1, space="PSUM"))
        sc = ph.enter_context(tc.tile_pool(name="sce", bufs=3))

        cfg = []
        for li, (wx_n, whh_n, cst) in enumerate(
            [("enc_wx", "enc_whh8", c_st), ("encp_wx", "encp_whh8", c2_st)]
        ):
            wx = ewp.tile([34, 2, G4], FP8, tag=f"wx{li}")
            nc.sync.dma_start(out=wx[:], in_=prm[wx_n][:])
            whh = []
            for k in range(4):
                t = ewp.tile([128, 2, G4], FP8, tag=f"ewhh{li}_{k}")
                nc.sync.dma_start(out=t[:], in_=prm[f"{whh_n}_{k}"][:])
                whh.append(t)
            xt = ewp.tile([34, T_IN * 2 * 128], FP8, tag=f"xt{li}")
            nc.sync.dma_start(
                out=xt[:].rearrange("p (t k b) -> p t k b", t=T_IN, k=2),
                in_=prm["xzT"][li].rearrange("t p k b -> p t k b"),
            )
            cfg.append((wx, whh, xt, cst))
        xt3 = [cfg[li][2][:].rearrange("p (t k b) -> p t k b", t=T_IN, k=2)
               for li in range(2)]

        h8prev = [None, None]
        hlast = [None, None]
        for t in range(T_IN):
            htiles = [
                sc.tile([BS, H], F16, tag=f"he{li}", name=f"he{li}_{t}")
                for li in range(2)
            ]
            h8new = [
                cpool.tile([BS, H], FP8, name="h8_dec")
                if (t == T_IN - 1 and li == 0)
                else sc.tile([BS, H], FP8, tag=f"h8{li}", name=f"h8{li}_{t}")
                for li in range(2)
            ]
            for j in range(2):  # gate halves [i|f|o|g]*512
                for li, (wx, whh, xt, cst) in enumerate(cfg):
                    skew = False
                    with tc.tile_wait_until(0.005, enable=skew):
                        ps = gp.tile([BS, HALF], F32, tag=f"g{li}",
                                     name=f"eg{li}_{t}_{j}")
                        for n in range(HALF // NMM):
                            co = j * HALF + n * NMM
                            nc.tensor.matmul(
                                ps[:, n * NMM:(n + 1) * NMM],
                                xt3[li][:, t, :, :],
                                wx[:, :, co:co + NMM],
                                start=True, stop=(t == 0),
                                perf_mode=PM.DoubleRow,
                            )
                            if t > 0:
                                h38 = h8prev[li][:].rearrange(
                                    "p (k b) -> p k b", k=8)
                                for kp in range(4):
                                    nc.tensor.matmul(
                                        ps[:, n * NMM:(n + 1) * NMM],
                                        h38[:, 2 * kp:2 * kp + 2, :],
                                        whh[kp][:, :, co:co + NMM],
                                        start=False, stop=(kp == 3),
                                        perf_mode=PM.DoubleRow,
                                    )
                        # evacuate half: [i|f|o|g]*512
                        hs = slice(512 * j, 512 * (j + 1))
                        slot = li * T_IN + t
                        scol = slot * H + 512 * j
                        with tc.high_priority(HIPRI):
                            sif = sc.tile([BS, 1024], F16, tag="esif")
                            so = sc.tile([BS, 512], F16, tag="eso")
                            tg = sc.tile([BS, 512], F16, tag="etg")
                            nc.scalar.activation(sif[:], ps[:, 0:1024],
                                                 AF.Sigmoid, scale=1.0 / S_EG)
                            nc.scalar.activation(tg[:], ps[:, 1536:2048],
                                                 AF.Tanh, scale=1.0 / S_EG)
                            nc.scalar.activation(so[:], ps[:, 1024:1536],
                                                 AF.Sigmoid, scale=1.0 / S_EG)
                            if t == 0:
                                nc.vector.tensor_mul(cst[:, hs], sif[:, 0:512],
                                                     tg[:])
                            else:
                                t2 = sc.tile([BS, 512], F16, tag="et2")
                                nc.vector.tensor_mul(t2[:], sif[:, 0:512], tg[:])
                                t1 = sc.tile([BS, 512], F16, tag="et1")
                                nc.gpsimd.tensor_mul(t1[:], sif[:, 512:1024],
                                                     cst[:, hs])
                                nc.vector.tensor_add(cst[:, hs], t1[:], t2[:])
                            tc_ = sc.tile([BS, 512], F16, tag="etc")
                            nc.scalar.activation(tc_[:], cst[:, hs], AF.Tanh)
                            h_cols = htiles[li][:, hs]
                            nc.vector.tensor_mul(h_cols, so[:], tc_[:])
                            nc.sync.dma_start_transpose(
                                catT[:, scol:scol + 512].rearrange(
                                    "p (k b) -> p k b", k=4),
                                h_cols,
                            )
                            nc.vector.tensor_scalar_mul(
                                h8new[li][:, 512 * j:512 * j + 256],
                                catT[:, scol:scol + 256], S_ACT)
                            nc.vector.tensor_scalar_mul(
                                h8new[li][:, 512 * j + 256:512 * j + 512],
                                catT[:, scol + 256:scol + 512], S_ACT)
                        nc.gpsimd.tensor_scalar_mul(
                            cat8[:, scol:scol + 512], h_cols, S_ACT)
            for li in range(2):
                h8prev[li] = h8new[li]
                hlast[li] = htiles[li]

        # decoder primes: h = enc h_9; catT block 20 = catT block 9
        nc.vector.tensor_copy(h_un[:], hlast[0][:])
        nc.vector.tensor_copy(
            catT[:, NSTAT * H:(NSTAT + 1) * H],
            catT[:, (T_IN - 1) * H:T_IN * H],
        )
        hT8_final = h8prev[0]

    # ---------------- decoder phase ---------------------------------------
    with ExitStack() as ph:
        dwp = ph.enter_context(tc.tile_pool(name="dwp", bufs=1))
        ident = dwp.tile([128, 128], F16)
        identS = dwp.tile([128, 128], F16)
        ones_row = dwp.tile([1, 128], F16)
        ones2 = dwp.tile([1, 2, 128], FP8)
        dbias8 = dwp.tile([1, 2, G4], FP8)
        nc.sync.dma_start(out=ident[:], in_=prm["ident"][:])
        nc.sync.dma_start(out=identS[:], in_=prm["identS"][:])
        nc.sync.dma_start(out=ones_row[:], in_=prm["ones_row"][:])
        nc.sync.dma_start(out=ones2[:], in_=prm["ones2"][:])
        nc.sync.dma_start(out=dbias8[:], in_=prm["dbias8"][:])
        dwhh8 = []
        W28 = []
        for i in range(4):
            t = dwp.tile([128, 2, G4], FP8, tag=f"dwhh{i}")
            nc.gpsimd.dma_start(out=t[:], in_=prm[f"dwhh8_{i}"][:])
            dwhh8.append(t)
            t = dwp.tile([128, 2, G4], FP8, tag=f"W28{i}")
            nc.gpsimd.dma_start(out=t[:], in_=prm[f"W28_{i}"][:])
            W28.append(t)
        posew = []
        for k in range(8):
            t = dwp.tile([128, P], F16, tag=f"pose{k}")
            nc.gpsimd.dma_start(out=t[:], in_=prm["pose_wT"][k * 128:(k + 1) * 128, :])
            posew.append(t)
        pbias = dwp.tile([1, P], F16)
        nc.sync.dma_start(out=pbias[:], in_=prm["pose_b"][:])

        sc = ph.enter_context(tc.tile_pool(name="scd", bufs=3))
        s1 = ph.enter_context(tc.tile_pool(name="s1d", bufs=3))
        prd = ph.enter_context(tc.tile_pool(name="prd", bufs=8))
        prp = ph.enter_context(tc.tile_pool(name="prp", bufs=6))
        att = ph.enter_context(tc.tile_pool(name="att", bufs=3))
        gq = ph.enter_context(tc.tile_pool(name="gq", bufs=2, space="PSUM"))
        atp = ph.enter_context(tc.tile_pool(name="atp", bufs=1, space="PSUM"))
        spp = ph.enter_context(tc.tile_pool(name="spp", bufs=1, space="PSUM"))
        pop = ph.enter_context(tc.tile_pool(name="pop", bufs=1, space="PSUM"))

        scps = spp.tile([BS, NSLOT], F32)
        po_ps = pop.tile([BS, P], F32)

        def emit_products(col0, ncols, n_dve, tagsuf):
            """Score products over h-cols [col0, col0+ncols); first n_dve
            slots (mod rotation) on DVE, rest on Pool."""
            out = []
            for s in range(NSLOT):
                src = catT[:, s * H + col0: s * H + col0 + ncols]
                dve = ((s + col0 // 256) % NSLOT) < n_dve
                if dve:
                    pt = prd.tile([128, ncols], F16, tag=f"ptd{ncols}",
                                  name=f"ptd_{tagsuf}_{s}")
                    nc.vector.tensor_mul(pt[:], src, cT[:, col0:col0 + ncols])
                else:
                    pt = prp.tile([128, ncols], F16, tag=f"ptp{ncols}",
                                  name=f"ptp_{tagsuf}_{s}")
                    nc.gpsimd.tensor_mul(pt[:], src, cT[:, col0:col0 + ncols])
                out.append((s, col0, ncols, pt))
            return out

        def emit_reduces(prods):
            for s, col0, ncols, pt in prods:
                for sl in range(ncols // 128):
                    slab = col0 // 128 + sl
                    nc.tensor.matmul(
                        scps[:][:, s:s + 1],
                        pt[:, sl * 128:(sl + 1) * 128],
                        ones_col[:],
                        start=(slab == 0), stop=(slab == 7),
                    )

        # ---- prime: cT + scores for step 0 from the encoder-final state ----
        for q in range(4):
            qs = slice(256 * q, 256 * (q + 1))
            nc.sync.dma_start_transpose(
                cT[:, qs].rearrange("p (k b) -> p k b", k=2), c_st[:, qs])
        for hf in range(2):
            prods = emit_products(hf * 512, 512, 10, f"pr{hf}")
            emit_reduces(prods)

        hT8 = hT8_final
        for step in range(T_OUT):
            # ---- PE: prefill gate chunks (bias + W2 from hT8) ----
            # per quarter two 512-col 1-bank PSUM chunks: A=[i|f]*256 B=[o|g]*256
            gps = []
            def prefill(q):
                h38 = hT8[:].rearrange("p (k b) -> p k b", k=8)
                ps = gq.tile([BS, QTR], F32, tag="gq", name=f"gq_{step}_{q}")
                for n in range(2):
                    co = q * QTR + n * NMM
                    osl = slice(n * NMM, (n + 1) * NMM)
                    nc.tensor.matmul(
                        ps[:, osl], ones2[:], dbias8[:, :, co:co + NMM],
                        start=True, stop=False, perf_mode=PM.DoubleRow,
                    )
                    for kp in range(4):
                        nc.tensor.matmul(
                            ps[:, osl],
                            h38[:, 2 * kp:2 * kp + 2, :],
                            W28[kp][:, :, co:co + NMM],
                            start=False, stop=False, perf_mode=PM.DoubleRow,
                        )
                return ps

            gps.append(prefill(0))
            gps.append(prefill(1))

            # ---- softmax over scps (scores are small: no max-sub needed) ----
            with tc.high_priority(HIPRI):
                num = s1.tile([BS, NSLOT], F32, tag="num")
                den = s1.tile([BS, NSLOT], F32, tag="den")
                nc.scalar.activation(num[:], scps[:], AF.Sigmoid)
                # sig(-x) = 1 - sig(x): avoids a second ACT op + hop
                nc.vector.tensor_scalar(
                    out=den[:], in0=num[:], scalar1=-1.0, scalar2=1.0,
                    op0=ALU.mult, op1=ALU.add,
                )
                rden = s1.tile([BS, NSLOT], F32, tag="rden")
                nc.vector.reciprocal(rden[:], den[:])
                wts = s1.tile([BS, NSLOT], F32, tag="wts")
                ssum = s1.tile([BS, 1], F32, tag="ssum")
                nc.vector.scalar_tensor_tensor(
                    out=wts[:], in0=num[:], scalar=1.0, in1=rden[:],
                    op0=ALU.mult, op1=ALU.mult, accum_out=ssum[:],
                )
                rcp = s1.tile([BS, 1], F32, tag="rcp")
                nc.vector.reciprocal(rcp[:], ssum[:])

                # ---- diag weights ----
                dg2s = []
                for p_i in range(10):
                    dg2 = att.tile([128, 2, 128], FP8, tag=f"dg2_{p_i}",
                                   name=f"dg2_{step}_{p_i}")
                    for half in range(2):
                        s = 2 * p_i + half
                        eng = nc.vector if s in DIAG_DVE else nc.gpsimd
                        eng.tensor_scalar(
                            out=dg2[:, half, :], in0=ident[:],
                            scalar1=wts[:, s:s + 1], scalar2=rcp[:],
                            op0=ALU.mult, op1=ALU.mult,
                        )
                    dg2s.append(dg2)
                dg10 = att.tile([128, 128], F16, tag="dg10", name=f"dg10_{step}")
                nc.gpsimd.tensor_scalar(
                    out=dg10[:], in0=identS[:],
                    scalar1=wts[:, NSTAT:NSLOT], scalar2=rcp[:],
                    op0=ALU.mult, op1=ALU.mult,
                )

            # ---- AV transposed: atT_ps[h128, b] += cat8(moving) x dg(stat) --
            atT8 = s1.tile([BS, H], FP8, tag="atT8")
            c83 = cat8[:].rearrange("p (s h) -> p s h", s=NSTAT)
            atps = atp.tile([BS, H], F32, tag="at", name=f"at_{step}")
            for hb in range(8):
                hsl = slice(hb * 128, (hb + 1) * 128)
                for p_i in range(10):
                    nc.tensor.matmul(
                        atps[:, hsl],
                        c83[:, 2 * p_i:2 * p_i + 2, hsl],
                        dg2s[p_i][:],
                        start=(p_i == 0), stop=False,
                        perf_mode=PM.DoubleRow,
                    )
                nc.tensor.matmul(
                    atps[:, hsl], h_un[:, hsl], dg10[:],
                    start=False, stop=True,
                )
                if hb == 3:
                    with tc.high_priority(HIPRI):
                        nc.scalar.activation(atT8[:, 0:512], atps[:, 0:512],
                                             AF.Copy)
                elif hb == 7:
                    with tc.high_priority(HIPRI):
                        nc.scalar.activation(atT8[:, 512:1024],
                                             atps[:, 512:1024], AF.Copy)

            # ---- gates: 4 PSUM quarters [i|f|o|g]*256 ----
            a3 = atT8[:].rearrange("p (k b) -> p k b", k=8)
            hT8_new = att.tile([BS, H], FP8, tag="hT8n", name=f"hT8_{step}")
            prods0 = prods1 = None

            def at_dr(q):
                ps = gps[q]
                for n in range(2):
                    co = q * QTR + n * NMM
                    osl = slice(n * NMM, (n + 1) * NMM)
                    for kp in range(4):
                        nc.tensor.matmul(
                            ps[:, osl],
                            a3[:, 2 * kp:2 * kp + 2, :],
                            dwhh8[kp][:, :, co:co + NMM],
                            start=False, stop=(kp == 3),
                            perf_mode=PM.DoubleRow,
                        )

            def evac(q):
                ps = gps[q]
                qs = slice(256 * q, 256 * (q + 1))
                with tc.high_priority(HIPRI):
                    sif = sc.tile([BS, 512], F16, tag="sif")
                    so = sc.tile([BS, 256], F16, tag="so")
                    tg = sc.tile([BS, 256], F16, tag="tg")
                    nc.scalar.activation(sif[:], ps[:, 0:512], AF.Sigmoid,
                                         scale=1.0 / S_G)
                    nc.scalar.activation(tg[:], ps[:, 768:1024], AF.Tanh,
                                         scale=1.0 / S_G)
                    nc.scalar.activation(so[:], ps[:, 512:768], AF.Sigmoid,
                                         scale=1.0 / S_G)
                    t2 = sc.tile([BS, 256], F16, tag="t2")
                    nc.gpsimd.tensor_mul(t2[:], sif[:, 0:256], tg[:])
                    t1 = sc.tile([BS, 256], F16, tag="t1")
                    nc.vector.tensor_mul(t1[:], sif[:, 256:512], c_st[:, qs])
                    nc.vector.tensor_add(c_st[:, qs], t1[:], t2[:])
                    tc_ = sc.tile([BS, 256], F16, tag="tc")
                    nc.scalar.activation(tc_[:], c_st[:, qs], AF.Tanh)
                    nc.vector.tensor_mul(h_un[:, qs], so[:], tc_[:])
                    hcol = NSTAT * H + 256 * q
                    nc.sync.dma_start_transpose(
                        cT[:, qs].rearrange("p (k b) -> p k b", k=2),
                        c_st[:, qs])
                    nc.sync.dma_start_transpose(
                        catT[:, hcol:hcol + 256].rearrange(
                            "p (k b) -> p k b", k=2),
                        h_un[:, qs],
                    )

            at_dr(0)
            evac(0)
            at_dr(1)
            evac(1)
            # half0 of next scores + hT8 half0
            nc.vector.tensor_scalar_mul(
                hT8_new[:, 0:512], catT[:, NSTAT * H:NSTAT * H + 512], S_ACT)
            prods0 = emit_products(0, 512, 14, f"s{step}h0")
            gps.append(prefill(2))
            at_dr(2)
            evac(2)
            gps.append(prefill(3))
            at_dr(3)
            evac(3)
            nc.scalar.activation(
                hT8_new[:, 512:1024],
                catT[:, NSTAT * H + 512:NSTAT * H + 1024], AF.Copy,
                scale=S_ACT)
            prods3 = emit_products(512, 512, 14, f"s{step}h1")
            hT8 = hT8_new

            # ---- pose output ----
            nc.tensor.matmul(po_ps[:], ones_row[:], pbias[:], start=True,
                             stop=False)
            for k in range(8):
                nc.tensor.matmul(
                    po_ps[:], catT[:, NSTAT * H + k * 128:NSTAT * H + (k + 1) * 128],
                    posew[k][:], start=False, stop=(k == 7),
                )
            stg = s1.tile([BS, P], F32, tag="stg")
            nc.scalar.activation(stg[:], po_ps[:], AF.Copy)
            nc.sync.dma_start(out=prm["out"][:, step, :], in_=stg[:])

            # ---- PE score reduces (tail) ----
            emit_reduces(prods0)
            emit_reduces(prods3)


_WAIT_LIMIT = {}
_WAIT_LIMIT_DEFAULT = 1


def _fix_waits(nc):
    """Hoist excess sync waits onto InstEventSemaphore carriers (the hardware
    instruction structs accept a limited number of wait commands)."""
    wid = 0
    for blk in nc.m.functions[0].blocks:
        insts = list(blk.instructions)
        out = []
        changed = False
        for inst in insts:
            si = getattr(inst, "sync_info", None)
            limit = _WAIT_LIMIT.get(type(inst).__name__, _WAIT_LIMIT_DEFAULT)
            if si is not None and len(si.on_wait) > limit:
                keep = si.on_wait[-limit:] if limit else []
                hoist = si.on_wait[: len(si.on_wait) - limit]
                for w in hoist:
                    carrier = mybir.InstEventSemaphore(
                        name=f"WFIX-{wid}",
                        engine=inst.engine,
                        ins=[],
                        outs=[],
                        sync_info=mybir.SyncInfo(on_wait=[w], on_update=[]),
                    )
                    wid += 1
                    out.append(carrier)
                inst.sync_info = mybir.SyncInfo(
                    on_wait=keep, on_update=list(si.on_update)
                )
                changed = True
            out.append(inst)
        if changed:
            blk.instructions = out


def build_nc(fix_waits=True):
    nc = bass.Bass()
    prm = {}
    decls = [
        ("ident", [128, 128], F16),
        ("identS", [128, 128], F16),
        ("ones_row", [1, 128], F16),
        ("ones2", [1, 2, 128], FP8),
        ("dbias8", [1, 2, G4], FP8),
        ("xzT", [2, T_IN, 34, 2, BS], FP8),
        ("enc_wx", [34, 2, G4], FP8),
        ("encp_wx", [34, 2, G4], FP8),
        ("pose_wT", [H, P], F16),
        ("pose_b", [1, P], F16),
    ]
    for i in range(4):
        decls.append((f"enc_whh8_{i}", [128, 2, G4], FP8))
        decls.append((f"encp_whh8_{i}", [128, 2, G4], FP8))
        decls.append((f"dwhh8_{i}", [128, 2, G4], FP8))
        decls.append((f"W28_{i}", [128, 2, G4], FP8))
    for name, shape, dt in decls:
        prm[name] = nc.declare_dram_parameter(name, shape, dt, isOutput=False)
    prm["out"] = nc.declare_dram_parameter("out", [BS, T_OUT, P], F32,
                                           isOutput=True)

    with ExitStack() as ctx:
        tc = ctx.enter_context(tile.TileContext(nc))
        _emit(ctx, nc, tc, prm)
    if fix_waits:
        _fix_waits(nc)
    return nc


def make_in_maps(inputs):
    shared, per_core = _preprocess(inputs)
    in_maps = []
    for c in range(NCORES):
        m = dict(shared)
        m["xzT"] = per_core[c]
        in_maps.append(m)
    return in_maps


def run(inputs, **kw):
    nc = build_nc()
    in_maps = make_in_maps(inputs)
    return run_bass_kernel_spmd(nc, in_maps, list(range(NCORES)), **kw)


def kernel(**inputs) -> np.ndarray:
    res = run(inputs)
    return np.concatenate(
        [res.results[c]["out"] for c in range(NCORES)], axis=0
    ).astype(np.float32)


if __name__ == "__main__":
    nc = build_nc()
    print("built ok")
